# revision 11
# baseline (speedup 1.0000x reference)
"""Distributed attention kernel for 8 TRN2 NeuronCores (v4).

Problem: B=2, S=2048, D=1024, H=16 heads (hd=64), no causal mask, no
scaling.  out = softmax((x@Wq) (x@Wk)^T) (x@Wv) @ Wp + biases.

Sharding: DP=2 over batch x TP=4 over heads.  Core c handles batch c//4
and heads 4*(c%4) .. 4*(c%4)+3.  Each core computes its 4 heads'
attention in 512-q-row chunks; per chunk, two small AllGathers (one per
2-head om tile, 128KB each) give every core in the group the full
[1024 hd, 512 q] normalized attention block, and each core runs c_proj
for its OWN 256-column slice of w_proj (column-parallel, no reduction).

All matmuls bf16 (host-side cast), f32 accumulation.  Softmax skips the
max-subtraction (scores are O(+-25), exp safe in f32); the denominator
comes free as the 65th row of the PV matmul via an appended ones-column
on V.

Scheduling notes:
- The TRN2 HAM duty-cycle governor clamps the PE clock under sustained
  high MAC activity, so the full-density projection matmuls (QKV,
  c_proj) are spread as fine-grained filler units between score groups
  instead of dense bursts.
- Scores PSUM tiles hold 3 k-tiles (3 banks); each exp covers
  [128, 1536] (ACT is the pacing engine in steady state).
- One accumulator per 2KB PSUM bank everywhere (matmul start=True
  zeroes the full bank).
- Biases fold into DVE copies; softmax reciprocal via
  reciprocal_approx_fast on a partition-0 copy of z.
- A dummy warmup AllGather at t~0 absorbs the ~15us first-collective
  trigger latency.
- The last chunk's c_proj runs two-pass (heads 0-1 matmuls while the
  final gather of heads 2-3 is on the wire).
"""

import sys

if "/opt/trn_rl_repo" not in sys.path:
    sys.path.insert(0, "/opt/trn_rl_repo")

import numpy as np
import ml_dtypes

import concourse.bass as bass
import concourse.mybir as mybir
from concourse import bacc
from concourse.tile import TileContext
from concourse.bass_utils import run_bass_kernel_spmd

BF16 = mybir.dt.bfloat16
F32 = mybir.dt.float32

B, S, D = 2, 2048, 1024
H = 16
HD = 64
TP = 4  # tensor-parallel group size (cores per batch)
HPC = H // TP  # heads per core = 4
QC = HPC * HD  # q (or k or v) columns per core = 256
OCW = D // TP  # c_proj output columns per core = 256
SQB = 512  # q chunk (free dim of scores/pv matmuls)
NJ = S // SQB  # 4 chunks
NT = S // 128  # 16 sk tiles
NK = D // 128  # 8 contraction tiles for the projections

_CACHE = {}


def build():
    nc = bacc.Bacc(num_devices=8)

    xT_ext = nc.declare_dram_parameter("xT", [D, S], BF16, isOutput=False)
    wqkv_ext = nc.declare_dram_parameter("wqkv", [D, 3 * QC], BF16, isOutput=False)
    bqk_ext = nc.declare_dram_parameter("bqk", [2 * QC, 1], F32, isOutput=False)
    bv_ext = nc.declare_dram_parameter("bv", [1, QC], F32, isOutput=False)
    wpc_ext = nc.declare_dram_parameter("wpc", [D, OCW], BF16, isOutput=False)
    bpc_ext = nc.declare_dram_parameter("bpc", [1, OCW], F32, isOutput=False)
    out_ext = nc.declare_dram_parameter("out", [S, OCW], BF16, isOutput=True)

    groups = [[0, 1, 2, 3], [4, 5, 6, 7]]
    ag_in = [
        [nc.dram_tensor(f"ag_in{j}_{p}", [128, SQB], BF16) for p in range(2)]
        for j in range(NJ)
    ]
    ag_out = [
        [nc.dram_tensor(f"ag_out{j}_{p}", [TP, 128, SQB], BF16) for p in range(2)]
        for j in range(NJ)
    ]
    warm_in = nc.dram_tensor("warm_in", [1, 64], BF16)
    warm_out = nc.dram_tensor("warm_out", [4, 64], BF16)

    with TileContext(nc) as tc:
        with (
            tc.tile_pool(name="persist", bufs=1) as persist,
            tc.tile_pool(name="expt_pool", bufs=3) as expt_pool,
            tc.tile_pool(name="ps", bufs=2, space="PSUM") as ps,
            tc.tile_pool(name="small", bufs=4) as small_pool,
            tc.tile_pool(name="ot", bufs=4) as ot_pool,
            tc.tile_pool(name="osb", bufs=6) as osb_pool,
            tc.tile_pool(name="attp", bufs=16) as att_pool,
        ):
            # ---- warmup collective (absorbs CC-core init latency) ----
            wrm = persist.tile([1, 64], BF16, tag="wrm", name="wrm")
            nc.vector.memset(wrm, 0.0)
            nc.sync.dma_start(out=warm_in[:, :], in_=wrm)
            nc.gpsimd.collective_compute(
                "AllGather",
                mybir.AluOpType.bypass,
                replica_groups=groups,
                ins=[warm_in.ap()],
                outs=[warm_out.ap()],
            )

            # ---- persistent loads (interleaved so QKV can start early) ----
            xt = []
            wt = []
            for k in range(NK):
                tw = persist.tile([128, 3 * QC], BF16, tag=f"wt{k}", name=f"wt{k}")
                nc.sync.dma_start(out=tw, in_=wqkv_ext[k * 128 : (k + 1) * 128, :])
                wt.append(tw)
                tx = persist.tile([128, S], BF16, tag=f"xt{k}", name=f"xt{k}")
                nc.sync.dma_start(out=tx, in_=xT_ext[k * 128 : (k + 1) * 128, :])
                xt.append(tx)
            bqk = []
            for k in range(4):
                t = persist.tile([128, 1], F32, tag=f"bqk{k}", name=f"bqk{k}")
                nc.sync.dma_start(out=t, in_=bqk_ext[k * 128 : (k + 1) * 128, :])
                bqk.append(t)
            bv = persist.tile([1, QC], F32, tag="bv", name="bv")
            nc.sync.dma_start(out=bv, in_=bv_ext[:, :])
            bpc = persist.tile([1, OCW], F32, tag="bpc", name="bpc")
            nc.sync.dma_start(out=bpc, in_=bpc_ext[:, :])
            wp = []
            for r in range(NK):
                t = persist.tile([128, OCW], BF16, tag=f"wp{r}", name=f"wp{r}")
                nc.sync.dma_start(out=t, in_=wpc_ext[r * 128 : (r + 1) * 128, :])
                wp.append(t)
            vb_b = persist.tile([128, QC], F32, tag="vb_b", name="vb_b")
            nc.gpsimd.partition_broadcast(vb_b, bv)
            pb_b = persist.tile([128, OCW], F32, tag="pb_b", name="pb_b")
            nc.gpsimd.partition_broadcast(pb_b, bpc)

            # v natural layout + ones column: v_sb[t] [128, HPC, 65]
            v_sb = []
            for t_i in range(NT):
                t = persist.tile(
                    [128, HPC, HD + 1], BF16, tag=f"v{t_i}", name=f"v{t_i}"
                )
                v_sb.append(t)
                nc.vector.memset(t[:, :, HD : HD + 1], 1.0)

            # q/k transposed layout: qk_sb[ct] [128, S]; ct 0-1 = q cols,
            # ct 2-3 = k cols; head h on partitions (h%2)*64 of tile h//2.
            qk_sb = [
                persist.tile([128, S], BF16, tag=f"qk{ct}", name=f"qk{ct}")
                for ct in range(4)
            ]

            # ---- wave A: k columns, k-major over 8 concurrent psums ----
            # (paced by the x/w DMA arrivals, so not a power burst)
            scA = [
                ps.tile([128, 3, SQB], F32, tag="sc", name=f"scA{i}") for i in range(2)
            ]
            pvA = [
                ps.tile([128, SQB], F32, tag="pv", name=f"pvA{i}") for i in range(2)
            ]
            wa = [(2, 0), (2, 1), (2, 2), (2, 3), (3, 0), (3, 1), (3, 2), (3, 3)]
            wa_aps = [
                scA[0][:, 0, :], scA[0][:, 1, :], scA[0][:, 2, :],
                scA[1][:, 0, :], scA[1][:, 1, :], scA[1][:, 2, :],
                pvA[0], pvA[1],
            ]
            for k in range(NK):
                for (ct, ns), ap in zip(wa, wa_aps):
                    nc.tensor.matmul(
                        ap,
                        wt[k][:, ct * 128 : (ct + 1) * 128],
                        xt[k][:, ns * SQB : (ns + 1) * SQB],
                        start=(k == 0),
                        stop=(k == NK - 1),
                    )
            for (ct, ns), ap in zip(wa, wa_aps):
                nc.vector.tensor_scalar_add(
                    qk_sb[ct][:, ns * SQB : (ns + 1) * SQB], ap, bqk[ct]
                )

            # ---- filler units (popped one per score group) ----
            fillers = []

            def pop_filler():
                if fillers:
                    fillers.pop(0)()

            def qcol_unit(ct, ns):
                t = ps.tile([128, 3, SQB], F32, tag="sc", name="qcols")
                for k in range(NK):
                    nc.tensor.matmul(
                        t[:, 0, :],
                        wt[k][:, ct * 128 : (ct + 1) * 128],
                        xt[k][:, ns * SQB : (ns + 1) * SQB],
                        start=(k == 0),
                        stop=(k == NK - 1),
                    )
                nc.vector.tensor_scalar_add(
                    qk_sb[ct][:, ns * SQB : (ns + 1) * SQB], t[:, 0, :], bqk[ct]
                )

            # q columns for chunk 0 must precede the first scores
            qcol_unit(0, 0)
            qcol_unit(1, 0)

            def v_unit(toks, tag):
                # one psum bank per accumulator (start=True zeroes the
                # whole 2KB zero region)
                if tag == "sc":
                    t = ps.tile([128, 3, SQB], F32, tag="sc", name="vB")
                    aps = [t[:, u, 0:256] for u in range(len(toks))]
                else:
                    t = [
                        ps.tile([128, SQB], F32, tag="pv", name="vP")
                        for _ in toks
                    ]
                    aps = [tt[:, 0:256] for tt in t]
                for k in range(NK):
                    for u, tt in enumerate(toks):
                        nc.tensor.matmul(
                            aps[u],
                            xt[k][:, tt * 128 : (tt + 1) * 128],
                            wt[k][:, 2 * QC : 3 * QC],
                            start=(k == 0),
                            stop=(k == NK - 1),
                        )
                for u, tt in enumerate(toks):
                    for hh in range(HPC):
                        nc.vector.tensor_add(
                            v_sb[tt][:, hh, 0:HD],
                            aps[u][:, hh * HD : (hh + 1) * HD],
                            vb_b[:, hh * HD : (hh + 1) * HD],
                        )

            # ---- attention pipeline ----
            def normalize(ph, pvp, pom):
                # copy z to a partition-0 tile first: the custom-DVE
                # reciprocal_approx_fast misreads inputs based at
                # partition 64 (native ops handle the shift fine)
                zrow = small_pool.tile([1, SQB], F32, tag="zrow", name="zrow")
                nc.vector.tensor_copy(zrow, pvp[HD : HD + 1, :])
                rz = small_pool.tile([1, SQB], F32, tag="rz", name="rz")
                nc.vector.reciprocal_approx_fast(rz, zrow)
                bc = small_pool.tile([HD, SQB], F32, tag="bc", name="bc")
                nc.gpsimd.partition_broadcast(bc, rz)
                if ph % 2 == 0:
                    nc.vector.tensor_mul(pom[ph // 2][0:HD, :], pvp[0:HD, :], bc)
                else:
                    o = ot_pool.tile([HD, SQB], BF16, tag="ot", name="ot")
                    nc.vector.tensor_mul(o, pvp[0:HD, :], bc)
                    nc.sync.dma_start(out=pom[ph // 2][HD:128, :], in_=o)

            GRP = [(0, 3), (3, 3), (6, 3), (9, 3), (12, 2), (14, 2)]

            def stage_ab(j, h, prev):
                expt = None
                if h is not None:
                    qslice = qk_sb[h // 2][
                        (h % 2) * HD : (h % 2) * HD + HD, j * SQB : (j + 1) * SQB
                    ]
                    krow = qk_sb[2 + h // 2][(h % 2) * HD : (h % 2) * HD + HD, :]
                    expt = expt_pool.tile(
                        [128, NT, SQB], BF16, tag="expt", name="expt"
                    )
                pvp = None
                if prev is not None:
                    pj, ph, pexpt, pom = prev
                    pvp = ps.tile([HD + 1, SQB], F32, tag="pv", name="pv")
                pv_t = [0]

                def emit_pv(n):
                    if prev is None:
                        return
                    while n > 0 and pv_t[0] < NT:
                        t_i = pv_t[0]
                        nc.tensor.matmul(
                            pvp,
                            v_sb[t_i][:, ph, :],
                            pexpt[:, t_i, :],
                            start=(t_i == 0),
                            stop=(t_i == NT - 1),
                        )
                        pv_t[0] += 1
                        n -= 1

                for t0, glen in GRP:
                    if h is not None:
                        ps_s = ps.tile([128, 3, SQB], F32, tag="sc", name="ps_s")
                        for u in range(glen):
                            nc.tensor.matmul(
                                ps_s[:, u, :],
                                krow[:, (t0 + u) * 128 : (t0 + u + 1) * 128],
                                qslice,
                                start=True,
                                stop=True,
                            )
                        nc.scalar.activation(
                            expt[:, t0 : t0 + glen, :],
                            ps_s[:, 0:glen, :],
                            mybir.ActivationFunctionType.Exp,
                        )
                    emit_pv(3)
                    pop_filler()
                emit_pv(NT)
                if prev is not None:
                    normalize(ph, pvp, pom)
                return expt

            # ---- allgather + column-parallel c_proj per chunk ----
            att_of = {}

            def send_ag(j, p, om):
                nc.sync.dma_start(out=ag_in[j][p][:, :], in_=om[p][:, :])
                nc.gpsimd.collective_compute(
                    "AllGather",
                    mybir.AluOpType.bypass,
                    replica_groups=groups,
                    ins=[ag_in[j][p].ap()],
                    outs=[ag_out[j][p].ap()],
                )

            def recv_att(j, p):
                tiles = []
                for sr in range(TP):
                    t = att_pool.tile(
                        [128, SQB], BF16, tag="attw", name=f"att{sr}_{p}"
                    )
                    nc.sync.dma_start(out=t, in_=ag_out[j][p][sr, :, :])
                    tiles.append(t)
                att_of[(j, p)] = tiles

            def cproj_unit(j, qq):
                # self-contained: alloc, 8 matmuls (p0 heads first), bias
                # fold, store
                pc_t = ps.tile([128, 3, SQB], F32, tag="sc", name="pc")
                pc = pc_t[:, 0, 0:OCW]
                for p in range(2):
                    for sr in range(TP):
                        nc.tensor.matmul(
                            pc,
                            att_of[(j, p)][sr][:, qq * 128 : (qq + 1) * 128],
                            wp[sr * 2 + p],
                            start=(p == 0 and sr == 0),
                            stop=(p == 1 and sr == TP - 1),
                        )
                osb = osb_pool.tile([128, OCW], BF16, tag="osb", name="osb")
                nc.vector.tensor_add(osb, pc, pb_b)
                nc.sync.dma_start(
                    out=out_ext[j * SQB + qq * 128 : j * SQB + (qq + 1) * 128, :],
                    in_=osb,
                )

            om_of = {}
            prev = None
            for j in range(NJ):
                om_of[j] = [
                    ot_pool.tile([128, SQB], BF16, tag="om", name=f"om{p}", bufs=4)
                    for p in range(2)
                ]
                for h in range(HPC):
                    if j == 0 and h == 0:
                        # v-projection units fill this stage's slots
                        fillers.extend(
                            [
                                lambda tk=tk: v_unit(tk, "sc")
                                for tk in ([0, 1, 2], [3, 4, 5], [6, 7, 8], [9, 10, 11])
                            ]
                            + [
                                lambda tk=tk: v_unit(tk, "pv")
                                for tk in ([12, 13], [14, 15])
                            ]
                        )
                    elif h == 1 and j + 1 < NJ:
                        fillers.extend(
                            [
                                lambda ct=ct, ns=j + 1: qcol_unit(ct, ns)
                                for ct in range(2)
                            ]
                        )
                    elif h == 3 and j >= 1:
                        fillers.extend(
                            [lambda q_=q_: cproj_unit(j - 1, q_) for q_ in range(4)]
                        )
                    expt = stage_ab(j, h, prev)
                    prev = (j, h, expt, om_of[j])
                    if h == 0 and j >= 1:
                        send_ag(j - 1, 1, om_of[j - 1])
                    elif h == 1 and j >= 1:
                        recv_att(j - 1, 0)
                    elif h == 2:
                        send_ag(j, 0, om_of[j])
                        if j >= 1:
                            recv_att(j - 1, 1)
            # drain: pv/normalize for the last head, then the last chunk's
            # second gather overlapped with the p0 half of its c_proj
            stage_ab(None, None, prev)
            j = NJ - 1
            send_ag(j, 1, om_of[j])
            recv_att(j, 0)
            t1 = ps.tile([128, 3, SQB], F32, tag="sc", name="pct1")
            t2 = ps.tile([128, 3, SQB], F32, tag="sc", name="pct2")
            pcs = [(t1 if qq < 3 else t2)[:, qq % 3, 0:OCW] for qq in range(4)]
            for qq in range(4):
                for sr in range(TP):
                    nc.tensor.matmul(
                        pcs[qq],
                        att_of[(j, 0)][sr][:, qq * 128 : (qq + 1) * 128],
                        wp[sr * 2],
                        start=(sr == 0),
                        stop=False,
                    )
            recv_att(j, 1)
            for qq in range(4):
                for sr in range(TP):
                    nc.tensor.matmul(
                        pcs[qq],
                        att_of[(j, 1)][sr][:, qq * 128 : (qq + 1) * 128],
                        wp[sr * 2 + 1],
                        start=False,
                        stop=(sr == TP - 1),
                    )
            for qq in range(4):
                osb = osb_pool.tile([128, OCW], BF16, tag="osb", name="osb")
                nc.vector.tensor_add(osb, pcs[qq], pb_b)
                nc.sync.dma_start(
                    out=out_ext[j * SQB + qq * 128 : j * SQB + (qq + 1) * 128, :],
                    in_=osb,
                )

    nc.compile()
    return nc


def make_in_maps(x, w_attn, b_attn, w_proj, b_proj):
    bf = ml_dtypes.bfloat16
    in_maps = []
    for c in range(8):
        b = c // TP
        g = c % TP
        cs = slice(g * QC, (g + 1) * QC)
        ocs = slice(g * OCW, (g + 1) * OCW)
        xT = np.ascontiguousarray(x[b].T).astype(bf)
        wqkv = np.concatenate(
            [w_attn[:, cs], w_attn[:, D:][:, cs], w_attn[:, 2 * D :][:, cs]], axis=1
        ).astype(bf)
        bqk = np.concatenate([b_attn[cs], b_attn[D:][cs]]).reshape(2 * QC, 1)
        bqk = np.ascontiguousarray(bqk, dtype=np.float32)
        bv = np.ascontiguousarray(
            b_attn[2 * D :][cs].reshape(1, QC), dtype=np.float32
        )
        wpc = np.ascontiguousarray(w_proj[:, ocs]).astype(bf)
        bpc = np.ascontiguousarray(b_proj[ocs].reshape(1, OCW), dtype=np.float32)
        in_maps.append(
            {"xT": xT, "wqkv": wqkv, "bqk": bqk, "bv": bv, "wpc": wpc, "bpc": bpc}
        )
    return in_maps


def assemble(results):
    # Core (b, g) owns output columns g*OCW..(g+1)*OCW for all of batch b.
    out = np.empty((B, S, D), np.float32)
    for c in range(8):
        b = c // TP
        g = c % TP
        o = np.asarray(results[c]["out"]).astype(np.float32)
        out[b, :, g * OCW : (g + 1) * OCW] = o
    return out


def kernel(x, w_attn, b_attn, w_proj, b_proj):
    x = np.asarray(x, dtype=np.float32)
    w_attn = np.asarray(w_attn, dtype=np.float32)
    b_attn = np.asarray(b_attn, dtype=np.float32)
    w_proj = np.asarray(w_proj, dtype=np.float32)
    b_proj = np.asarray(b_proj, dtype=np.float32)
    if "nc" not in _CACHE:
        _CACHE["nc"] = build()
    nc = _CACHE["nc"]
    in_maps = make_in_maps(x, w_attn, b_attn, w_proj, b_proj)
    res = run_bass_kernel_spmd(nc, in_maps, core_ids=list(range(8)))
    return assemble(res.results)


# revision 14
# speedup vs baseline: 1.0079x; 1.0079x over previous
"""Distributed attention kernel for 8 TRN2 NeuronCores (v4).

Problem: B=2, S=2048, D=1024, H=16 heads (hd=64), no causal mask, no
scaling.  out = softmax((x@Wq) (x@Wk)^T) (x@Wv) @ Wp + biases.

Sharding: DP=2 over batch x TP=4 over heads.  Core c handles batch c//4
and heads 4*(c%4) .. 4*(c%4)+3.  Each core computes its 4 heads'
attention in 512-q-row chunks; per chunk, two small AllGathers (one per
2-head om tile, 128KB each) give every core in the group the full
[1024 hd, 512 q] normalized attention block, and each core runs c_proj
for its OWN 256-column slice of w_proj (column-parallel, no reduction).

All matmuls bf16 (host-side cast), f32 accumulation.  Softmax skips the
max-subtraction (scores are O(+-25), exp safe in f32); the denominator
comes free as the 65th row of the PV matmul via an appended ones-column
on V.

Scheduling notes:
- The TRN2 HAM duty-cycle governor clamps the PE clock under sustained
  high MAC activity, so the full-density projection matmuls (QKV,
  c_proj) are spread as fine-grained filler units between score groups
  instead of dense bursts.
- Scores PSUM tiles hold 3 k-tiles (3 banks); each exp covers
  [128, 1536] (ACT is the pacing engine in steady state).
- One accumulator per 2KB PSUM bank everywhere (matmul start=True
  zeroes the full bank).
- Biases fold into DVE copies; softmax reciprocal via
  reciprocal_approx_fast on a partition-0 copy of z.
- A dummy warmup AllGather at t~0 absorbs the ~15us first-collective
  trigger latency.
- The last chunk's c_proj runs two-pass (heads 0-1 matmuls while the
  final gather of heads 2-3 is on the wire).
"""

import sys

if "/opt/trn_rl_repo" not in sys.path:
    sys.path.insert(0, "/opt/trn_rl_repo")

import numpy as np
import ml_dtypes

import concourse.bass as bass
import concourse.mybir as mybir
from concourse import bacc
from concourse.tile import TileContext
from concourse.bass_utils import run_bass_kernel_spmd

BF16 = mybir.dt.bfloat16
F32 = mybir.dt.float32

B, S, D = 2, 2048, 1024
H = 16
HD = 64
TP = 4  # tensor-parallel group size (cores per batch)
HPC = H // TP  # heads per core = 4
QC = HPC * HD  # q (or k or v) columns per core = 256
OCW = D // TP  # c_proj output columns per core = 256
SQB = 512  # q chunk (free dim of scores/pv matmuls)
NJ = S // SQB  # 4 chunks
NT = S // 128  # 16 sk tiles
NK = D // 128  # 8 contraction tiles for the projections

_CACHE = {}


def build():
    nc = bacc.Bacc(num_devices=8)

    xT_ext = nc.declare_dram_parameter("xT", [D, S], BF16, isOutput=False)
    wqkv_ext = nc.declare_dram_parameter("wqkv", [D, 3 * QC], BF16, isOutput=False)
    bqk_ext = nc.declare_dram_parameter("bqk", [2 * QC, 1], F32, isOutput=False)
    bv_ext = nc.declare_dram_parameter("bv", [1, QC], F32, isOutput=False)
    wpc_ext = nc.declare_dram_parameter("wpc", [D, OCW], BF16, isOutput=False)
    bpc_ext = nc.declare_dram_parameter("bpc", [1, OCW], F32, isOutput=False)
    out_ext = nc.declare_dram_parameter("out", [S, OCW], BF16, isOutput=True)

    groups = [[0, 1, 2, 3], [4, 5, 6, 7]]
    ag_in = [
        [nc.dram_tensor(f"ag_in{j}_{p}", [128, SQB], BF16) for p in range(2)]
        for j in range(NJ)
    ]
    ag_out = [
        [nc.dram_tensor(f"ag_out{j}_{p}", [TP, 128, SQB], BF16) for p in range(2)]
        for j in range(NJ)
    ]
    warm_in = nc.dram_tensor("warm_in", [1, 64], BF16)
    warm_out = nc.dram_tensor("warm_out", [4, 64], BF16)
    agh_in = [nc.dram_tensor(f"agh_in{i}", [HD, SQB], BF16) for i in range(2)]
    agh_out = [nc.dram_tensor(f"agh_out{i}", [TP, HD, SQB], BF16) for i in range(2)]

    with TileContext(nc) as tc:
        with (
            tc.tile_pool(name="persist", bufs=1) as persist,
            tc.tile_pool(name="expt_pool", bufs=3) as expt_pool,
            tc.tile_pool(name="ps", bufs=2, space="PSUM") as ps,
            tc.tile_pool(name="small", bufs=4) as small_pool,
            tc.tile_pool(name="ot", bufs=4) as ot_pool,
            tc.tile_pool(name="osb", bufs=6) as osb_pool,
            tc.tile_pool(name="attp", bufs=16) as att_pool,
        ):
            # ---- warmup collective (absorbs CC-core init latency) ----
            wrm = persist.tile([1, 64], BF16, tag="wrm", name="wrm")
            nc.vector.memset(wrm, 0.0)
            nc.sync.dma_start(out=warm_in[:, :], in_=wrm)
            nc.gpsimd.collective_compute(
                "AllGather",
                mybir.AluOpType.bypass,
                replica_groups=groups,
                ins=[warm_in.ap()],
                outs=[warm_out.ap()],
            )

            # ---- persistent loads (interleaved so QKV can start early) ----
            xt = []
            wt = []
            for k in range(NK):
                tw = persist.tile([128, 3 * QC], BF16, tag=f"wt{k}", name=f"wt{k}")
                nc.sync.dma_start(out=tw, in_=wqkv_ext[k * 128 : (k + 1) * 128, :])
                wt.append(tw)
                tx = persist.tile([128, S], BF16, tag=f"xt{k}", name=f"xt{k}")
                nc.sync.dma_start(out=tx, in_=xT_ext[k * 128 : (k + 1) * 128, :])
                xt.append(tx)
            bqk = []
            for k in range(4):
                t = persist.tile([128, 1], F32, tag=f"bqk{k}", name=f"bqk{k}")
                nc.sync.dma_start(out=t, in_=bqk_ext[k * 128 : (k + 1) * 128, :])
                bqk.append(t)
            bv = persist.tile([1, QC], F32, tag="bv", name="bv")
            nc.sync.dma_start(out=bv, in_=bv_ext[:, :])
            bpc = persist.tile([1, OCW], F32, tag="bpc", name="bpc")
            nc.sync.dma_start(out=bpc, in_=bpc_ext[:, :])
            wp = []
            for r in range(NK):
                t = persist.tile([128, OCW], BF16, tag=f"wp{r}", name=f"wp{r}")
                nc.sync.dma_start(out=t, in_=wpc_ext[r * 128 : (r + 1) * 128, :])
                wp.append(t)
            vb_b = persist.tile([128, QC], F32, tag="vb_b", name="vb_b")
            nc.gpsimd.partition_broadcast(vb_b, bv)
            pb_b = persist.tile([128, OCW], F32, tag="pb_b", name="pb_b")
            nc.gpsimd.partition_broadcast(pb_b, bpc)

            # v natural layout + ones column: v_sb[t] [128, HPC, 65]
            v_sb = []
            for t_i in range(NT):
                t = persist.tile(
                    [128, HPC, HD + 1], BF16, tag=f"v{t_i}", name=f"v{t_i}"
                )
                v_sb.append(t)
                nc.vector.memset(t[:, :, HD : HD + 1], 1.0)

            # q/k transposed layout: qk_sb[ct] [128, S]; ct 0-1 = q cols,
            # ct 2-3 = k cols; head h on partitions (h%2)*64 of tile h//2.
            qk_sb = [
                persist.tile([128, S], BF16, tag=f"qk{ct}", name=f"qk{ct}")
                for ct in range(4)
            ]

            # ---- wave A: k columns, k-major over 8 concurrent psums ----
            # (paced by the x/w DMA arrivals, so not a power burst)
            scA = [
                ps.tile([128, 2, SQB], F32, tag="sc", name=f"scA{i}") for i in range(3)
            ]
            pvA = [
                ps.tile([128, SQB], F32, tag="pv", name=f"pvA{i}") for i in range(2)
            ]
            wa = [(2, 0), (2, 1), (2, 2), (2, 3), (3, 0), (3, 1), (3, 2), (3, 3)]
            wa_aps = [
                scA[0][:, 0, :], scA[0][:, 1, :],
                scA[1][:, 0, :], scA[1][:, 1, :],
                scA[2][:, 0, :], scA[2][:, 1, :],
                pvA[0], pvA[1],
            ]
            for k in range(NK):
                for (ct, ns), ap in zip(wa, wa_aps):
                    nc.tensor.matmul(
                        ap,
                        wt[k][:, ct * 128 : (ct + 1) * 128],
                        xt[k][:, ns * SQB : (ns + 1) * SQB],
                        start=(k == 0),
                        stop=(k == NK - 1),
                    )
            for (ct, ns), ap in zip(wa, wa_aps):
                nc.vector.tensor_scalar_add(
                    qk_sb[ct][:, ns * SQB : (ns + 1) * SQB], ap, bqk[ct]
                )

            # ---- filler units (popped one per score group) ----
            fillers = []

            def pop_filler():
                if fillers:
                    fillers.pop(0)()

            def qcol_unit(ct, ns):
                t = ps.tile([128, 2, SQB], F32, tag="sc", name="qcols")
                for k in range(NK):
                    nc.tensor.matmul(
                        t[:, 0, :],
                        wt[k][:, ct * 128 : (ct + 1) * 128],
                        xt[k][:, ns * SQB : (ns + 1) * SQB],
                        start=(k == 0),
                        stop=(k == NK - 1),
                    )
                nc.vector.tensor_scalar_add(
                    qk_sb[ct][:, ns * SQB : (ns + 1) * SQB], t[:, 0, :], bqk[ct]
                )

            # q columns for chunk 0 must precede the first scores
            qcol_unit(0, 0)
            qcol_unit(1, 0)

            def v_unit(toks, tag):
                # one psum bank per accumulator (start=True zeroes the
                # whole 2KB zero region)
                if tag == "sc":
                    t = ps.tile([128, 2, SQB], F32, tag="sc", name="vB")
                    aps = [t[:, u, 0:256] for u in range(len(toks))]
                else:
                    t = [
                        ps.tile([128, SQB], F32, tag="pv", name="vP")
                        for _ in toks
                    ]
                    aps = [tt[:, 0:256] for tt in t]
                for k in range(NK):
                    for u, tt in enumerate(toks):
                        nc.tensor.matmul(
                            aps[u],
                            xt[k][:, tt * 128 : (tt + 1) * 128],
                            wt[k][:, 2 * QC : 3 * QC],
                            start=(k == 0),
                            stop=(k == NK - 1),
                        )
                for u, tt in enumerate(toks):
                    for hh in range(HPC):
                        nc.vector.tensor_add(
                            v_sb[tt][:, hh, 0:HD],
                            aps[u][:, hh * HD : (hh + 1) * HD],
                            vb_b[:, hh * HD : (hh + 1) * HD],
                        )

            # ---- attention pipeline ----
            def normalize(ph, pvp, pom):
                # copy z to a partition-0 tile first: the custom-DVE
                # reciprocal_approx_fast misreads inputs based at
                # partition 64 (native ops handle the shift fine)
                zrow = small_pool.tile([1, SQB], F32, tag="zrow", name="zrow")
                nc.vector.tensor_copy(zrow, pvp[HD : HD + 1, :])
                rz = small_pool.tile([1, SQB], F32, tag="rz", name="rz")
                nc.vector.reciprocal_approx_fast(rz, zrow)
                bc = small_pool.tile([HD, SQB], F32, tag="bc", name="bc")
                nc.gpsimd.partition_broadcast(bc, rz)
                if ph % 2 == 0:
                    nc.vector.tensor_mul(pom[ph // 2][0:HD, :], pvp[0:HD, :], bc)
                else:
                    o = ot_pool.tile([HD, SQB], BF16, tag="ot", name="ot")
                    nc.vector.tensor_mul(o, pvp[0:HD, :], bc)
                    nc.sync.dma_start(out=pom[ph // 2][HD:128, :], in_=o)

            GRP = [(t, 2) for t in range(0, NT, 2)]

            def stage_ab(j, h, prev):
                expt = None
                if h is not None:
                    qslice = qk_sb[h // 2][
                        (h % 2) * HD : (h % 2) * HD + HD, j * SQB : (j + 1) * SQB
                    ]
                    krow = qk_sb[2 + h // 2][(h % 2) * HD : (h % 2) * HD + HD, :]
                    expt = expt_pool.tile(
                        [128, NT, SQB], BF16, tag="expt", name="expt"
                    )
                pvp = None
                if prev is not None:
                    pj, ph, pexpt, pom = prev
                    pvp = ps.tile([HD + 1, SQB], F32, tag="pv", name="pv")
                pv_t = [0]

                def emit_pv(n):
                    if prev is None:
                        return
                    while n > 0 and pv_t[0] < NT:
                        t_i = pv_t[0]
                        nc.tensor.matmul(
                            pvp,
                            v_sb[t_i][:, ph, :],
                            pexpt[:, t_i, :],
                            start=(t_i == 0),
                            stop=(t_i == NT - 1),
                        )
                        pv_t[0] += 1
                        n -= 1

                for t0, glen in GRP:
                    if h is not None:
                        ps_s = ps.tile([128, 2, SQB], F32, tag="sc", name="ps_s")
                        for u in range(glen):
                            nc.tensor.matmul(
                                ps_s[:, u, :],
                                krow[:, (t0 + u) * 128 : (t0 + u + 1) * 128],
                                qslice,
                                start=True,
                                stop=True,
                            )
                        nc.scalar.activation(
                            expt[:, t0 : t0 + glen, :],
                            ps_s[:, 0:glen, :],
                            mybir.ActivationFunctionType.Exp,
                        )
                    emit_pv(2)
                    pop_filler()
                emit_pv(NT)
                if prev is not None:
                    normalize(ph, pvp, pom)
                return expt

            # ---- allgather + column-parallel c_proj per chunk ----
            att_of = {}

            def send_ag(j, p, om):
                nc.sync.dma_start(out=ag_in[j][p][:, :], in_=om[p][:, :])
                nc.gpsimd.collective_compute(
                    "AllGather",
                    mybir.AluOpType.bypass,
                    replica_groups=groups,
                    ins=[ag_in[j][p].ap()],
                    outs=[ag_out[j][p].ap()],
                )

            def recv_att(j, p):
                tiles = []
                for sr in range(TP):
                    t = att_pool.tile(
                        [128, SQB], BF16, tag="attw", name=f"att{sr}_{p}"
                    )
                    nc.sync.dma_start(out=t, in_=ag_out[j][p][sr, :, :])
                    tiles.append(t)
                att_of[(j, p)] = tiles

            def cproj_unit(j, qq):
                # self-contained: alloc, 8 matmuls (p0 heads first), bias
                # fold, store
                pc_t = ps.tile([128, 2, SQB], F32, tag="sc", name="pc")
                pc = pc_t[:, 0, 0:OCW]
                for p in range(2):
                    for sr in range(TP):
                        nc.tensor.matmul(
                            pc,
                            att_of[(j, p)][sr][:, qq * 128 : (qq + 1) * 128],
                            wp[sr * 2 + p],
                            start=(p == 0 and sr == 0),
                            stop=(p == 1 and sr == TP - 1),
                        )
                osb = osb_pool.tile([128, OCW], BF16, tag="osb", name="osb")
                nc.vector.tensor_add(osb, pc, pb_b)
                nc.sync.dma_start(
                    out=out_ext[j * SQB + qq * 128 : j * SQB + (qq + 1) * 128, :],
                    in_=osb,
                )

            om_of = {}
            prev = None
            for j in range(NJ):
                om_of[j] = [
                    ot_pool.tile([128, SQB], BF16, tag="om", name=f"om{p}", bufs=4)
                    for p in range(2)
                ]
                for h in range(HPC):
                    if j == 0 and h == 0:
                        # v-projection units fill this stage's slots
                        fillers.extend(
                            [
                                lambda tk=tk: v_unit(tk, "sc")
                                for tk in (
                                    [0, 1], [2, 3], [4, 5], [6, 7],
                                    [8, 9], [10, 11],
                                )
                            ]
                            + [
                                lambda tk=tk: v_unit(tk, "pv")
                                for tk in ([12, 13], [14, 15])
                            ]
                        )
                    elif h == 1 and j + 1 < NJ:
                        fillers.extend(
                            [
                                lambda ct=ct, ns=j + 1: qcol_unit(ct, ns)
                                for ct in range(2)
                            ]
                        )
                    elif h == 3 and j >= 1:
                        fillers.extend(
                            [lambda q_=q_: cproj_unit(j - 1, q_) for q_ in range(4)]
                        )
                    expt = stage_ab(j, h, prev)
                    prev = (j, h, expt, om_of[j])
                    if h == 0 and j >= 1:
                        send_ag(j - 1, 1, om_of[j - 1])
                    elif h == 1 and j >= 1:
                        recv_att(j - 1, 0)
                    elif h == 2:
                        send_ag(j, 0, om_of[j])
                        if j >= 1:
                            recv_att(j - 1, 1)
            # ---- tail: last chunk, head-split gathers ----
            # head 2's om half is ready after stage (3,3); ship it before
            # the drain so only head 3's 64KB gather sits in the tail,
            # overlapped with the other heads' c_proj matmuls.
            j = NJ - 1
            nc.sync.dma_start(out=agh_in[0][:, :], in_=om_of[j][1][0:HD, :])
            nc.gpsimd.collective_compute(
                "AllGather",
                mybir.AluOpType.bypass,
                replica_groups=groups,
                ins=[agh_in[0].ap()],
                outs=[agh_out[0].ap()],
            )
            stage_ab(None, None, prev)  # pv + normalize for head 3
            nc.sync.dma_start(out=agh_in[1][:, :], in_=om_of[j][1][HD:128, :])
            nc.gpsimd.collective_compute(
                "AllGather",
                mybir.AluOpType.bypass,
                replica_groups=groups,
                ins=[agh_in[1].ap()],
                outs=[agh_out[1].ap()],
            )
            recv_att(j, 0)
            t1 = ps.tile([128, 2, SQB], F32, tag="sc", name="pct1")
            t2 = ps.tile([128, 2, SQB], F32, tag="sc", name="pct2")
            pcs = [(t1 if qq < 2 else t2)[:, qq % 2, 0:OCW] for qq in range(4)]
            for qq in range(4):
                for sr in range(TP):
                    nc.tensor.matmul(
                        pcs[qq],
                        att_of[(j, 0)][sr][:, qq * 128 : (qq + 1) * 128],
                        wp[sr * 2],
                        start=(sr == 0),
                        stop=False,
                    )
            th2 = []
            for sr in range(TP):
                t = att_pool.tile([HD, SQB], BF16, tag="atth", name=f"h2_{sr}", bufs=8)
                nc.sync.dma_start(out=t, in_=agh_out[0][sr, :, :])
                th2.append(t)
            for qq in range(4):
                for sr in range(TP):
                    nc.tensor.matmul(
                        pcs[qq],
                        th2[sr][:, qq * 128 : (qq + 1) * 128],
                        wp[sr * 2 + 1][0:HD, :],
                        start=False,
                        stop=False,
                    )
            th3 = []
            for sr in range(TP):
                t = att_pool.tile([128, SQB], BF16, tag="atth", name=f"h3_{sr}", bufs=8)
                nc.sync.dma_start(out=t[HD:128, :], in_=agh_out[1][sr, :, :])
                th3.append(t)
            for qq in range(4):
                for sr in range(TP):
                    nc.tensor.matmul(
                        pcs[qq],
                        th3[sr][HD:128, qq * 128 : (qq + 1) * 128],
                        wp[sr * 2 + 1][HD:128, :],
                        start=False,
                        stop=(sr == TP - 1),
                    )
            for qq in range(4):
                osb = osb_pool.tile([128, OCW], BF16, tag="osb", name="osb")
                nc.vector.tensor_add(osb, pcs[qq], pb_b)
                nc.sync.dma_start(
                    out=out_ext[j * SQB + qq * 128 : j * SQB + (qq + 1) * 128, :],
                    in_=osb,
                )

    nc.compile()
    return nc


def make_in_maps(x, w_attn, b_attn, w_proj, b_proj):
    bf = ml_dtypes.bfloat16
    in_maps = []
    for c in range(8):
        b = c // TP
        g = c % TP
        cs = slice(g * QC, (g + 1) * QC)
        ocs = slice(g * OCW, (g + 1) * OCW)
        xT = np.ascontiguousarray(x[b].T).astype(bf)
        wqkv = np.concatenate(
            [w_attn[:, cs], w_attn[:, D:][:, cs], w_attn[:, 2 * D :][:, cs]], axis=1
        ).astype(bf)
        bqk = np.concatenate([b_attn[cs], b_attn[D:][cs]]).reshape(2 * QC, 1)
        bqk = np.ascontiguousarray(bqk, dtype=np.float32)
        bv = np.ascontiguousarray(
            b_attn[2 * D :][cs].reshape(1, QC), dtype=np.float32
        )
        wpc = np.ascontiguousarray(w_proj[:, ocs]).astype(bf)
        bpc = np.ascontiguousarray(b_proj[ocs].reshape(1, OCW), dtype=np.float32)
        in_maps.append(
            {"xT": xT, "wqkv": wqkv, "bqk": bqk, "bv": bv, "wpc": wpc, "bpc": bpc}
        )
    return in_maps


def assemble(results):
    # Core (b, g) owns output columns g*OCW..(g+1)*OCW for all of batch b.
    out = np.empty((B, S, D), np.float32)
    for c in range(8):
        b = c // TP
        g = c % TP
        o = np.asarray(results[c]["out"]).astype(np.float32)
        out[b, :, g * OCW : (g + 1) * OCW] = o
    return out


def kernel(x, w_attn, b_attn, w_proj, b_proj):
    x = np.asarray(x, dtype=np.float32)
    w_attn = np.asarray(w_attn, dtype=np.float32)
    b_attn = np.asarray(b_attn, dtype=np.float32)
    w_proj = np.asarray(w_proj, dtype=np.float32)
    b_proj = np.asarray(b_proj, dtype=np.float32)
    if "nc" not in _CACHE:
        _CACHE["nc"] = build()
    nc = _CACHE["nc"]
    in_maps = make_in_maps(x, w_attn, b_attn, w_proj, b_proj)
    res = run_bass_kernel_spmd(nc, in_maps, core_ids=list(range(8)))
    return assemble(res.results)
